# revision 21
# baseline (speedup 1.0000x reference)
"""Local/global multihead attention on 8 NeuronCores (Trainium2, Bass/Tile).

Sharding: core c = b*2 + hg  (b = batch 0..3, hg = head-group 0/1, 8 heads each).
Each core computes q/k/v projections for its 8 heads on its batch, head-local
attention (slot 0 runs a dense 2048-key path driven by a per-core mask so the
SPMD program is uniform: hg0's slot 0 is the true global head with an all-ones
mask, hg1's slot 0 is a local head with a band mask), banded attention with
narrowed tq windows for slots 1-7, and the output projection restricted to its
head-group columns of wo. Host sums the two head-group partials per batch and
adds bo + bv @ wo.T (valid because softmax rows sum to 1; bk is dropped
entirely since exp(q.bk) cancels in softmax).

v2 notes vs v1:
- softmax denominators via vector.reciprocal_approx_fast (custom DVE op,
  ~5x faster than InstReciprocal which measured 4us per [1,512]).
- banded score windows packed into one [128,1536] at tile per (h,s): one
  mask multiply instead of six.
- q bias folded into the scalar-engine PSUM->SBUF copy (activation bias);
  k bias dropped (softmax-invariant).
- q/k projections loop-reordered so 4 matmuls share one LoadStationary.
- out-projection PSUM->SBUF copies moved to gpsimd.
- software pipelining: v-projection chunks and AV matmuls staggered inside
  the dense jc loop; banded (h,s) stages staggered by one.

All matmul operands are bf16 (TensorE runs 1 cyc/row vs 4 for fp32); PSUM
accumulation is fp32 throughout.
"""
import numpy as np
import ml_dtypes

E, H, D, LK = 1024, 16, 64, 128
SCALE = D ** -0.5
B, N = 4, 2048
FG = 512          # features per head-group (8 heads * 64)
NCORES = 8

# narrowed tq windows per dj variant (delta = (dj-1)*128)
WIN = [(0, 128), (0, 256), (0, 384), (128, 512), (256, 512), (384, 512)]
# packed column offsets of each dj window inside the [128,1536] at tile
POS = [0, 128, 384, 768, 1152, 1408]
# psum packing: P0=[dj0,dj1] P1=[dj2] P2=[dj3] P3=[dj4,dj5]
PGRP = [(0, 0), (0, 128), (1, 0), (2, 0), (3, 0), (3, 256)]  # (ptile, col0)

_cache = {}


def _bf16(a):
    return np.ascontiguousarray(a.astype(ml_dtypes.bfloat16))


def _build():
    import concourse.bacc as bacc
    import concourse.tile as tile
    import concourse.mybir as mybir
    from concourse.bass import ts

    dt = mybir.dt
    AF = mybir.ActivationFunctionType

    nc = bacc.Bacc("TRN2", target_bir_lowering=False, debug=False,
                   num_devices=NCORES)

    xT = nc.dram_tensor("xT", [E, N], dt.bfloat16, kind="ExternalInput")
    wqT = nc.dram_tensor("wqT", [E, FG], dt.bfloat16, kind="ExternalInput")
    wkT = nc.dram_tensor("wkT", [E, FG], dt.bfloat16, kind="ExternalInput")
    wvT = nc.dram_tensor("wvT", [E, FG], dt.bfloat16, kind="ExternalInput")
    woT = nc.dram_tensor("woT", [FG, E], dt.bfloat16, kind="ExternalInput")
    bqc = nc.dram_tensor("bqc", [128, 4], dt.float32, kind="ExternalInput")
    lmask = nc.dram_tensor("lmask", [128, 1536], dt.bfloat16, kind="ExternalInput")
    gmask = nc.dram_tensor("gmask", [16, 128, N], dt.bfloat16, kind="ExternalInput")
    out = nc.dram_tensor("out", [N, E], dt.bfloat16, kind="ExternalOutput")

    with tile.TileContext(nc) as tc:
        with (
            tc.tile_pool(name="wts", bufs=1) as wts,
            tc.tile_pool(name="xp", bufs=1) as xp,
            tc.tile_pool(name="qkv", bufs=1) as qkv,
            tc.tile_pool(name="att", bufs=3) as att,
            tc.tile_pool(name="gm", bufs=2) as gm,
            tc.tile_pool(name="small", bufs=4) as small,
            tc.tile_pool(name="ps", bufs=1, space="PSUM") as psp,
        ):
            # ---- load weights/x/masks (ordered so qk_proj(0) starts ASAP) ----
            xT_t = [xp.tile([128, N], dt.bfloat16, name=f"xT{i}", tag=f"xT{i}") for i in range(8)]
            wq_t = [wts.tile([128, FG], dt.bfloat16, name=f"wq{i}", tag=f"wq{i}") for i in range(8)]
            wk_t = [wts.tile([128, FG], dt.bfloat16, name=f"wk{i}", tag=f"wk{i}") for i in range(8)]
            wv_t = [wts.tile([128, FG], dt.bfloat16, name=f"wv{i}", tag=f"wv{i}") for i in range(8)]
            bq_t = small.tile([128, 4], dt.float32, name="bq", tag="bq")
            nc.sync.dma_start(bq_t[:], bqc[:, :])
            # weights ride the Activation-engine DGE queue so they stream in
            # parallel with xT on the SP queue
            for ec in range(8):
                nc.sync.dma_start(xT_t[ec][:], xT[ts(ec, 128), :])
                nc.scalar.dma_start(wq_t[ec][:], wqT[ts(ec, 128), :])
                nc.scalar.dma_start(wk_t[ec][:], wkT[ts(ec, 128), :])
            for ec in range(8):
                nc.scalar.dma_start(wv_t[ec][:], wvT[ts(ec, 128), :])
            lm_t = wts.tile([128, 1536], dt.bfloat16, name="lm", tag="lm")
            nc.scalar.dma_start(lm_t[:], lmask[:, :])
            wo_t = [wts.tile([128, E], dt.bfloat16, name=f"wo{i}", tag=f"wo{i}") for i in range(4)]
            for fc in range(4):
                nc.scalar.dma_start(wo_t[fc][:], woT[ts(fc, 128), :])

            qT_sb = [qkv.tile([128, N], dt.bfloat16, name=f"qT{i}", tag=f"qT{i}") for i in range(4)]
            kT_sb = [qkv.tile([128, N], dt.bfloat16, name=f"kT{i}", tag=f"kT{i}") for i in range(4)]
            v_sb = [qkv.tile([128, 8 * 72], dt.bfloat16, name=f"v{i}", tag=f"v{i}") for i in range(16)]
            outTn = [qkv.tile([128, N], dt.bfloat16, name=f"outTn{i}", tag=f"outTn{i}") for i in range(4)]

            def qk_proj(fc):
                """project q and k feature chunk fc (128 rows of qT/kT)."""
                for dst, w_t, biased in ((qT_sb, wq_t, True), (kT_sb, wk_t, False)):
                    accs = [psp.tile([128, 512], dt.float32, name=f"acc{t}",
                                     tag=f"acc{t}") for t in range(4)]
                    for ec in range(8):
                        for tcn in range(4):
                            nc.tensor.matmul(
                                accs[tcn][:], w_t[ec][:, ts(fc, 128)],
                                xT_t[ec][:, ts(tcn, 512)],
                                start=(ec == 0), stop=(ec == 7))
                    for tcn in range(4):
                        if biased:
                            nc.scalar.activation(
                                dst[fc][:, ts(tcn, 512)], accs[tcn][:],
                                AF.Identity, bias=bq_t[:, fc:fc + 1])
                        else:
                            nc.vector.tensor_copy(dst[fc][:, ts(tcn, 512)],
                                                  accs[tcn][:])

            def qk_proj2(fc):
                """fc 1-3, emitted inside the dense loop: psum pairs on the
                (still idle) banded-AV banks, copies on vector so the scalar
                engine stays free for the dense exps."""
                for dst, w_t, biased in ((qT_sb, wq_t, True), (kT_sb, wk_t, False)):
                    for p0 in (0, 2):
                        accs = [psp.tile([128, 512], dt.float32, name="qacc",
                                         tag="av", bufs=2) for _ in range(2)]
                        for ec in range(8):
                            for j in range(2):
                                nc.tensor.matmul(
                                    accs[j][:], w_t[ec][:, ts(fc, 128)],
                                    xT_t[ec][:, ts(p0 + j, 512)],
                                    start=(ec == 0), stop=(ec == 7))
                        for j in range(2):
                            if biased:
                                nc.vector.tensor_scalar_add(
                                    dst[fc][:, ts(p0 + j, 512)], accs[j][:],
                                    bq_t[:, fc:fc + 1])
                            else:
                                nc.vector.tensor_copy(
                                    dst[fc][:, ts(p0 + j, 512)], accs[j][:])

            def v_proj(tcn):
                """v chunk tcn: natural layout, per-head 72-col strided + ones col."""
                ps = psp.tile([128, 512], dt.float32, name="ps", tag="ps", bufs=2)
                for ec in range(8):
                    nc.tensor.matmul(ps[:], xT_t[ec][:, ts(tcn, 128)], wv_t[ec][:],
                                     start=(ec == 0), stop=(ec == 7))
                src = ps[:].rearrange("p (h d) -> p h d", h=8)
                dst = v_sb[tcn][:].rearrange("p (h d) -> p h d", h=8)[:, :, 0:64]
                nc.vector.tensor_copy(dst, src)
                ones = v_sb[tcn][:].rearrange("p (h d) -> p h d", h=8)[:, :, 64:65]
                nc.vector.memset(ones, 1.0)

            def head_rows(t, h):
                r0 = (h % 2) * 64
                return t[h // 2][r0:r0 + 64, :]

            def divide(h, s, av):
                """outTn rows for (h, s-block) = av numerators / denominator."""
                den = small.tile([1, 512], dt.float32, name="den", tag="den")
                nc.vector.tensor_copy(den[:], av[64:65, :])
                rec = small.tile([1, 512], dt.float32, name="rec", tag="rec")
                nc.vector.reciprocal_approx_fast(out=rec[:], in_=den[:])
                rec64 = small.tile([64, 512], dt.float32, name="rec64", tag="rec64")
                nc.gpsimd.partition_broadcast(rec64[:], rec[:])
                nc.vector.tensor_mul(head_rows(outTn, h)[:, ts(s, 512)],
                                     av[0:64, :], rec64[:])

            # ================== emission ==================
            qk_proj(0)

            # ---- slot 0: dense 2048-key path with gmask, jc-outer.
            # Stage jc emits: v-proj chunk jc, gmask DMA, QK+exp+mask for jc,
            # then the AV for jc-1 (stagger keeps PE busy while scalar works).
            h = 0
            qh = head_rows(qT_sb, h)
            kh = head_rows(kT_sb, h)
            av_g = [psp.tile([128, 512], dt.float32, name=f"avg{t}",
                             tag=f"acc{t}") for t in range(4)]
            g_at = [None] * 16
            for jc in range(16):
                gt = gm.tile([128, N], dt.bfloat16, name="gm", tag="gm")
                nc.sync.dma_start(gt[:], gmask[jc, :, :])
                at = att.tile([128, N], dt.bfloat16, name="gat", tag="gat")
                g_at[jc] = at
                for s in range(4):
                    ps = psp.tile([128, 512], dt.float32, name="ps", tag="ps", bufs=2)
                    nc.tensor.matmul(ps[:], kh[:, ts(jc, 128)], qh[:, ts(s, 512)],
                                     start=True, stop=True)
                    nc.scalar.activation(at[:, ts(s, 512)], ps[:], AF.Exp,
                                         scale=float(SCALE))
                v_proj(jc)
                nc.vector.tensor_mul(at[:], at[:], gt[:])
                if jc > 0:
                    for s in range(4):
                        nc.tensor.matmul(
                            av_g[s][0:65, :], v_sb[jc - 1][:, h * 72:h * 72 + 65],
                            g_at[jc - 1][:, ts(s, 512)], start=(jc - 1 == 0),
                            stop=False, skip_group_check=True)
            for s in range(4):
                nc.tensor.matmul(
                    av_g[s][0:65, :], v_sb[15][:, h * 72:h * 72 + 65],
                    g_at[15][:, ts(s, 512)], start=False, stop=True,
                    skip_group_check=True)
            for s in range(4):
                divide(0, s, av_g[s])

            # ---- slots 1..7: banded path, s-major so each 512-token block's
            # output projection interleaves as soon as its last head divides ----
            stages = []
            for s in range(4):
                for h in range(1, 8):
                    stages.append((h, s))

            def outproj_block(s):
                for tcn in range(4 * s, 4 * s + 4):
                    t0 = (tcn % 2) * 2
                    pss = [psp.tile([128, 512], dt.float32, name=f"ops{oc}",
                                    tag=f"acc{t0 + oc}") for oc in range(2)]
                    for fc in range(4):
                        for oc in range(2):
                            nc.tensor.matmul(pss[oc][:],
                                             outTn[fc][:, ts(tcn, 128)],
                                             wo_t[fc][:, ts(oc, 512)],
                                             start=(fc == 0), stop=(fc == 3))
                    for oc in range(2):
                        ob = att.tile([128, 512], dt.bfloat16, name="ob", tag="ob")
                        nc.vector.tensor_copy(ob[:], pss[oc][:])
                        nc.sync.dma_start(out[ts(tcn, 128), ts(oc, 512)], ob[:])

            pend = []  # [(h, s, av, at, djs)] awaiting AV emission

            def emit_qk(h, s):
                qh = head_rows(qT_sb, h)
                kh = head_rows(kT_sb, h)
                djs = [dj for dj in range(6) if 0 <= s * 4 - 1 + dj <= 15]
                at = att.tile([128, 1536], dt.bfloat16, name="at", tag="at")
                ptiles = [None] * 4
                for dj in djs:
                    pt, pc0 = PGRP[dj]
                    if ptiles[pt] is None:
                        ptiles[pt] = psp.tile([128, 512], dt.float32, name="bps",
                                              tag="ps", bufs=2)
                    jc = s * 4 - 1 + dj
                    c0, c1 = WIN[dj]
                    w = c1 - c0
                    nc.tensor.matmul(ptiles[pt][:, pc0:pc0 + w], kh[:, ts(jc, 128)],
                                     qh[:, s * 512 + c0:s * 512 + c1],
                                     start=True, stop=True, skip_group_check=True)
                # exps: one per packed psum tile, into packed at positions
                done = set()
                for dj in djs:
                    pt, pc0 = PGRP[dj]
                    if pt in done:
                        continue
                    done.add(pt)
                    # full extent of this ptile used by djs present
                    lo = min(PGRP[d][1] for d in djs if PGRP[d][0] == pt)
                    hi = max(PGRP[d][1] + WIN[d][1] - WIN[d][0]
                             for d in djs if PGRP[d][0] == pt)
                    atlo = min(POS[d] for d in djs if PGRP[d][0] == pt)
                    nc.scalar.activation(at[:, atlo:atlo + (hi - lo)],
                                         ptiles[pt][:, lo:hi], AF.Exp,
                                         scale=float(SCALE))
                # one mask multiply over the packed tile
                lo = min(POS[d] for d in djs)
                hi = max(POS[d] + WIN[d][1] - WIN[d][0] for d in djs)
                nc.vector.tensor_mul(at[:, lo:hi], at[:, lo:hi], lm_t[:, lo:hi])
                av = psp.tile([128, 512], dt.float32, name="av", tag="av", bufs=2)
                return av, at, djs

            def emit_av(h, s, av, at, djs):
                for i, dj in enumerate(djs):
                    jc = s * 4 - 1 + dj
                    c0, c1 = WIN[dj]
                    nc.tensor.matmul(
                        av[0:65, c0:c1], v_sb[jc][:, h * 72:h * 72 + 65],
                        at[:, POS[dj]:POS[dj] + (c1 - c0)],
                        start=(i == 0), stop=(i == len(djs) - 1),
                        skip_group_check=True)

            def retire(entry):
                ph, psn, pav, pat, pdjs = entry
                emit_av(ph, psn, pav, pat, pdjs)
                divide(ph, psn, pav)
                if ph == 7:
                    outproj_block(psn)

            for h, s in stages:
                if s == 0 and h in (2, 4, 6):
                    qk_proj(h // 2)
                av, at, djs = emit_qk(h, s)
                pend.append((h, s, av, at, djs))
                if len(pend) > 1:
                    retire(pend.pop(0))
            while pend:
                retire(pend.pop(0))
    nc.finalize()
    return nc


def _host_inputs(x, wq, bq, wk, bk, wv, bv, wo, bo):
    """Build the 8 per-core input dicts."""
    r = np.arange(128)[:, None]
    lm = np.zeros((128, 1536), np.float32)
    for dj, ((c0, c1), pos) in enumerate(zip(WIN, POS)):
        c = np.arange(c0, c1)[None, :]
        lm[:, pos:pos + (c1 - c0)] = (np.abs((dj - 1) * 128 + r - c) <= LK)
    lm = _bf16(lm)

    cN = np.arange(N)[None, :]
    gm_band = np.zeros((16, 128, N), np.float32)
    for jc in range(16):
        gm_band[jc] = (np.abs(128 * jc + r - cN) <= LK)
    gm_ones = _bf16(np.ones((16, 128, N), np.float32))
    gm_band = _bf16(gm_band)

    in_maps = []
    for core in range(NCORES):
        b, hg = core // 2, core % 2
        fsl = slice(hg * FG, (hg + 1) * FG)
        in_maps.append({
            "xT": _bf16(x[b].T),
            "wqT": _bf16(wq[fsl].T),
            "wkT": _bf16(wk[fsl].T),
            "wvT": _bf16(wv[fsl].T),
            "woT": _bf16(wo[:, fsl].T),
            "bqc": np.ascontiguousarray(bq[fsl].reshape(4, 128).T, np.float32),
            "lmask": lm,
            "gmask": gm_ones if hg == 0 else gm_band,
        })
    return in_maps


def kernel(x, wq, bq, wk, bk, wv, bv, wo, bo):
    from concourse.bass_utils import run_bass_kernel_spmd

    x, wq, bq, wk, bk, wv, bv, wo, bo = (
        np.asarray(a, np.float32) for a in (x, wq, bq, wk, bk, wv, bv, wo, bo))

    if "nc" not in _cache:
        _cache["nc"] = _build()
    nc = _cache["nc"]

    in_maps = _host_inputs(x, wq, bq, wk, bk, wv, bv, wo, bo)
    res = run_bass_kernel_spmd(nc, in_maps, core_ids=list(range(NCORES)))
    _cache["last_results"] = res

    const = (bo + bv @ wo.T).astype(np.float32)        # [1024]
    out = np.empty((B, N, E), np.float32)
    for b in range(B):
        out[b] = (np.asarray(res.results[2 * b]["out"], np.float32)
                  + np.asarray(res.results[2 * b + 1]["out"], np.float32)
                  + const)
    return out


# revision 26
# speedup vs baseline: 1.0947x; 1.0947x over previous
"""Local/global multihead attention on 8 NeuronCores (Trainium2, Bass/Tile).

Sharding: core c = b*2 + hg  (b = batch 0..3, hg = head-group 0/1, 8 heads each).
Each core computes q/k/v projections for its 8 heads on its batch, head-local
attention (slot 0 runs a dense 2048-key path driven by a per-core mask so the
SPMD program is uniform: hg0's slot 0 is the true global head with an all-ones
mask, hg1's slot 0 is a local head with a band mask), banded attention with
narrowed tq windows for slots 1-7, and the output projection restricted to its
head-group columns of wo. Host sums the two head-group partials per batch and
adds bo + bv @ wo.T (valid because softmax rows sum to 1; bk is dropped
entirely since exp(q.bk) cancels in softmax).

v2 notes vs v1:
- softmax denominators via vector.reciprocal_approx_fast (custom DVE op,
  ~5x faster than InstReciprocal which measured 4us per [1,512]).
- banded score windows packed into one [128,1536] at tile per (h,s): one
  mask multiply instead of six.
- q bias folded into the scalar-engine PSUM->SBUF copy (activation bias);
  k bias dropped (softmax-invariant).
- q/k projections loop-reordered so 4 matmuls share one LoadStationary.
- out-projection PSUM->SBUF copies moved to gpsimd.
- software pipelining: v-projection chunks and AV matmuls staggered inside
  the dense jc loop; banded (h,s) stages staggered by one.

All matmul operands are bf16 (TensorE runs 1 cyc/row vs 4 for fp32); PSUM
accumulation is fp32 throughout.
"""
import numpy as np
import ml_dtypes

E, H, D, LK = 1024, 16, 64, 128
SCALE = D ** -0.5
B, N = 4, 2048
FG = 512          # features per head-group (8 heads * 64)
NCORES = 8

# narrowed tq windows per dj variant (delta = (dj-1)*128)
WIN = [(0, 128), (0, 256), (0, 384), (128, 512), (256, 512), (384, 512)]
# packed column offsets of each dj window inside the [128,1536] at tile
POS = [0, 128, 384, 768, 1152, 1408]
# psum packing: P0=[dj0,dj1] P1=[dj2] P2=[dj3] P3=[dj4,dj5]
PGRP = [(0, 0), (0, 128), (1, 0), (2, 0), (3, 0), (3, 256)]  # (ptile, col0)

_cache = {}


def _bf16(a):
    return np.ascontiguousarray(a.astype(ml_dtypes.bfloat16))


def _build():
    import concourse.bacc as bacc
    import concourse.tile as tile
    import concourse.mybir as mybir
    from concourse.bass import ts

    dt = mybir.dt
    AF = mybir.ActivationFunctionType

    nc = bacc.Bacc("TRN2", target_bir_lowering=False, debug=False,
                   num_devices=NCORES)

    xT = nc.dram_tensor("xT", [E, N], dt.bfloat16, kind="ExternalInput")
    wqT = nc.dram_tensor("wqT", [E, FG], dt.bfloat16, kind="ExternalInput")
    wkT = nc.dram_tensor("wkT", [E, FG], dt.bfloat16, kind="ExternalInput")
    wvT = nc.dram_tensor("wvT", [E, FG], dt.bfloat16, kind="ExternalInput")
    woT = nc.dram_tensor("woT", [FG, E], dt.bfloat16, kind="ExternalInput")
    bqc = nc.dram_tensor("bqc", [128, 4], dt.float32, kind="ExternalInput")
    lmask = nc.dram_tensor("lmask", [128, 1536], dt.bfloat16, kind="ExternalInput")
    gmask = nc.dram_tensor("gmask", [16, 128, N], dt.bfloat16, kind="ExternalInput")
    out = nc.dram_tensor("out", [N, E], dt.bfloat16, kind="ExternalOutput")

    with tile.TileContext(nc) as tc:
        with (
            tc.tile_pool(name="wts", bufs=1) as wts,
            tc.tile_pool(name="xp", bufs=1) as xp,
            tc.tile_pool(name="qkv", bufs=1) as qkv,
            tc.tile_pool(name="att", bufs=3) as att,
            tc.tile_pool(name="gm", bufs=2) as gm,
            tc.tile_pool(name="small", bufs=4) as small,
            tc.tile_pool(name="ps", bufs=1, space="PSUM") as psp,
        ):
            # ---- load weights/x/masks (ordered so qk_proj(0) starts ASAP) ----
            xT_t = [xp.tile([128, N], dt.bfloat16, name=f"xT{i}", tag=f"xT{i}") for i in range(8)]
            wq_t = [wts.tile([128, FG], dt.bfloat16, name=f"wq{i}", tag=f"wq{i}") for i in range(8)]
            wk_t = [wts.tile([128, FG], dt.bfloat16, name=f"wk{i}", tag=f"wk{i}") for i in range(8)]
            wv_t = [wts.tile([128, FG], dt.bfloat16, name=f"wv{i}", tag=f"wv{i}") for i in range(8)]
            bq_t = small.tile([128, 4], dt.float32, name="bq", tag="bq")
            nc.sync.dma_start(bq_t[:], bqc[:, :])
            for ec in range(8):
                nc.sync.dma_start(xT_t[ec][:], xT[ts(ec, 128), :])
                nc.sync.dma_start(wq_t[ec][:], wqT[ts(ec, 128), :])
                nc.sync.dma_start(wk_t[ec][:], wkT[ts(ec, 128), :])
            for ec in range(8):
                nc.sync.dma_start(wv_t[ec][:], wvT[ts(ec, 128), :])
            lm_t = wts.tile([128, 1536], dt.bfloat16, name="lm", tag="lm")
            nc.sync.dma_start(lm_t[:], lmask[:, :])
            wo_t = [wts.tile([128, E], dt.bfloat16, name=f"wo{i}", tag=f"wo{i}") for i in range(4)]
            for fc in range(4):
                nc.sync.dma_start(wo_t[fc][:], woT[ts(fc, 128), :])

            qT_sb = [qkv.tile([128, N], dt.bfloat16, name=f"qT{i}", tag=f"qT{i}") for i in range(4)]
            kT_sb = [qkv.tile([128, N], dt.bfloat16, name=f"kT{i}", tag=f"kT{i}") for i in range(4)]
            v_sb = [qkv.tile([128, 8 * 72], dt.bfloat16, name=f"v{i}", tag=f"v{i}") for i in range(16)]
            outTn = [qkv.tile([128, N], dt.bfloat16, name=f"outTn{i}", tag=f"outTn{i}") for i in range(4)]

            def qk_proj(fc):
                """project q and k feature chunk fc (128 rows of qT/kT)."""
                for dst, w_t, biased in ((qT_sb, wq_t, True), (kT_sb, wk_t, False)):
                    accs = [psp.tile([128, 512], dt.float32, name=f"acc{t}",
                                     tag=f"acc{t}") for t in range(4)]
                    for ec in range(8):
                        for tcn in range(4):
                            nc.tensor.matmul(
                                accs[tcn][:], w_t[ec][:, ts(fc, 128)],
                                xT_t[ec][:, ts(tcn, 512)],
                                start=(ec == 0), stop=(ec == 7))
                    for tcn in range(4):
                        if biased:
                            nc.scalar.activation(
                                dst[fc][:, ts(tcn, 512)], accs[tcn][:],
                                AF.Identity, bias=bq_t[:, fc:fc + 1])
                        else:
                            nc.vector.tensor_copy(dst[fc][:, ts(tcn, 512)],
                                                  accs[tcn][:])

            def v_proj(tcn):
                """v chunk tcn: natural layout, per-head 72-col strided + ones col."""
                ps = psp.tile([128, 512], dt.float32, name="ps", tag="ps", bufs=2)
                for ec in range(8):
                    nc.tensor.matmul(ps[:], xT_t[ec][:, ts(tcn, 128)], wv_t[ec][:],
                                     start=(ec == 0), stop=(ec == 7))
                src = ps[:].rearrange("p (h d) -> p h d", h=8)
                dst = v_sb[tcn][:].rearrange("p (h d) -> p h d", h=8)[:, :, 0:64]
                nc.vector.tensor_copy(dst, src)
                ones = v_sb[tcn][:].rearrange("p (h d) -> p h d", h=8)[:, :, 64:65]
                nc.vector.memset(ones, 1.0)

            def head_rows(t, h):
                r0 = (h % 2) * 64
                return t[h // 2][r0:r0 + 64, :]

            def divide(h, s, av):
                """outTn rows for (h, s-block) = av numerators / denominator."""
                den = small.tile([1, 512], dt.float32, name="den", tag="den")
                nc.vector.tensor_copy(den[:], av[64:65, :])
                rec = small.tile([1, 512], dt.float32, name="rec", tag="rec")
                nc.vector.reciprocal_approx_fast(out=rec[:], in_=den[:])
                rec64 = small.tile([64, 512], dt.float32, name="rec64", tag="rec64")
                nc.gpsimd.partition_broadcast(rec64[:], rec[:])
                nc.vector.tensor_mul(head_rows(outTn, h)[:, ts(s, 512)],
                                     av[0:64, :], rec64[:])

            # ================== emission ==================
            qk_proj(0)

            # ---- slot 0: dense 2048-key path with gmask, jc-outer.
            # Stage jc emits: v-proj chunk jc, gmask DMA, QK+exp+mask for jc,
            # then the AV for jc-1 (stagger keeps PE busy while scalar works).
            h = 0
            qh = head_rows(qT_sb, h)
            kh = head_rows(kT_sb, h)
            av_g = [psp.tile([128, 512], dt.float32, name=f"avg{t}",
                             tag=f"acc{t}") for t in range(4)]
            g_at = [None] * 16
            for jc in range(16):
                gt = gm.tile([128, N], dt.bfloat16, name="gm", tag="gm")
                nc.sync.dma_start(gt[:], gmask[jc, :, :])
                at = att.tile([128, N], dt.bfloat16, name="gat", tag="gat", bufs=4)
                g_at[jc] = at
                for s in range(4):
                    ps = psp.tile([128, 512], dt.float32, name="ps", tag="ps", bufs=2)
                    nc.tensor.matmul(ps[:], kh[:, ts(jc, 128)], qh[:, ts(s, 512)],
                                     start=True, stop=True)
                    nc.scalar.activation(at[:, ts(s, 512)], ps[:], AF.Exp,
                                         scale=float(SCALE))
                v_proj(jc)
                nc.vector.tensor_mul(at[:], at[:], gt[:])
                if jc > 0:
                    for s in range(4):
                        nc.tensor.matmul(
                            av_g[s][0:65, :], v_sb[jc - 1][:, h * 72:h * 72 + 65],
                            g_at[jc - 1][:, ts(s, 512)], start=(jc - 1 == 0),
                            stop=False, skip_group_check=True)
            for s in range(4):
                nc.tensor.matmul(
                    av_g[s][0:65, :], v_sb[15][:, h * 72:h * 72 + 65],
                    g_at[15][:, ts(s, 512)], start=False, stop=True,
                    skip_group_check=True)
            for s in range(4):
                divide(0, s, av_g[s])

            # ---- slots 1..7: banded path, (h,s) stages staggered by one ----
            stages = []
            for h in range(1, 8):
                for s in range(4):
                    stages.append((h, s))

            pend = []  # [(h, s, av, at, djs)] awaiting AV emission

            def outproj_block(s):
                """output projection for token block s (4 tcn chunks)."""
                for tcn in range(4 * s, 4 * s + 4):
                    t0 = (tcn % 2) * 2
                    pss = [psp.tile([128, 512], dt.float32, name=f"ops{oc}",
                                    tag=f"acc{t0 + oc}") for oc in range(2)]
                    for fc in range(4):
                        for oc in range(2):
                            nc.tensor.matmul(pss[oc][:],
                                             outTn[fc][:, ts(tcn, 128)],
                                             wo_t[fc][:, ts(oc, 512)],
                                             start=(fc == 0), stop=(fc == 3))
                    for oc in range(2):
                        ob = att.tile([128, 512], dt.bfloat16, name="ob", tag="ob")
                        nc.vector.tensor_copy(ob[:], pss[oc][:])
                        nc.sync.dma_start(out[ts(tcn, 128), ts(oc, 512)], ob[:])

            def emit_qk(h, s):
                qh = head_rows(qT_sb, h)
                kh = head_rows(kT_sb, h)
                djs = [dj for dj in range(6) if 0 <= s * 4 - 1 + dj <= 15]
                at = att.tile([128, 1536], dt.bfloat16, name="at", tag="at", bufs=4)
                ptiles = [None] * 4
                for dj in djs:
                    pt, pc0 = PGRP[dj]
                    if ptiles[pt] is None:
                        ptiles[pt] = psp.tile([128, 512], dt.float32, name="bps",
                                              tag="ps", bufs=2)
                    jc = s * 4 - 1 + dj
                    c0, c1 = WIN[dj]
                    w = c1 - c0
                    nc.tensor.matmul(ptiles[pt][:, pc0:pc0 + w], kh[:, ts(jc, 128)],
                                     qh[:, s * 512 + c0:s * 512 + c1],
                                     start=True, stop=True, skip_group_check=True)
                # exps: one per packed psum tile, into packed at positions
                done = set()
                for dj in djs:
                    pt, pc0 = PGRP[dj]
                    if pt in done:
                        continue
                    done.add(pt)
                    # full extent of this ptile used by djs present
                    lo = min(PGRP[d][1] for d in djs if PGRP[d][0] == pt)
                    hi = max(PGRP[d][1] + WIN[d][1] - WIN[d][0]
                             for d in djs if PGRP[d][0] == pt)
                    atlo = min(POS[d] for d in djs if PGRP[d][0] == pt)
                    nc.scalar.activation(at[:, atlo:atlo + (hi - lo)],
                                         ptiles[pt][:, lo:hi], AF.Exp,
                                         scale=float(SCALE))
                # one mask multiply over the packed tile
                lo = min(POS[d] for d in djs)
                hi = max(POS[d] + WIN[d][1] - WIN[d][0] for d in djs)
                nc.vector.tensor_mul(at[:, lo:hi], at[:, lo:hi], lm_t[:, lo:hi])
                av = psp.tile([128, 512], dt.float32, name="av", tag="av", bufs=2)
                return av, at, djs

            def emit_av(h, s, av, at, djs):
                for i, dj in enumerate(djs):
                    jc = s * 4 - 1 + dj
                    c0, c1 = WIN[dj]
                    nc.tensor.matmul(
                        av[0:65, c0:c1], v_sb[jc][:, h * 72:h * 72 + 65],
                        at[:, POS[dj]:POS[dj] + (c1 - c0)],
                        start=(i == 0), stop=(i == len(djs) - 1),
                        skip_group_check=True)

            def retire(entry):
                ph, psn, pav, pat, pdjs = entry
                emit_av(ph, psn, pav, pat, pdjs)
                divide(ph, psn, pav)
                if ph == 7:
                    # head 7 is the last writer of outTn token block psn:
                    # its output projection can stream out now, overlapping
                    # the remaining banded stages
                    outproj_block(psn)

            for h, s in stages:
                if h >= 2 and s == 0 and h % 2 == 0:
                    qk_proj(h // 2)
                av, at, djs = emit_qk(h, s)
                pend.append((h, s, av, at, djs))
                if len(pend) > 1:
                    retire(pend.pop(0))
            while pend:
                retire(pend.pop(0))
    nc.finalize()
    return nc


def _host_inputs(x, wq, bq, wk, bk, wv, bv, wo, bo):
    """Build the 8 per-core input dicts."""
    r = np.arange(128)[:, None]
    lm = np.zeros((128, 1536), np.float32)
    for dj, ((c0, c1), pos) in enumerate(zip(WIN, POS)):
        c = np.arange(c0, c1)[None, :]
        lm[:, pos:pos + (c1 - c0)] = (np.abs((dj - 1) * 128 + r - c) <= LK)
    lm = _bf16(lm)

    cN = np.arange(N)[None, :]
    gm_band = np.zeros((16, 128, N), np.float32)
    for jc in range(16):
        gm_band[jc] = (np.abs(128 * jc + r - cN) <= LK)
    gm_ones = _bf16(np.ones((16, 128, N), np.float32))
    gm_band = _bf16(gm_band)

    in_maps = []
    for core in range(NCORES):
        b, hg = core // 2, core % 2
        fsl = slice(hg * FG, (hg + 1) * FG)
        in_maps.append({
            "xT": _bf16(x[b].T),
            "wqT": _bf16(wq[fsl].T),
            "wkT": _bf16(wk[fsl].T),
            "wvT": _bf16(wv[fsl].T),
            "woT": _bf16(wo[:, fsl].T),
            "bqc": np.ascontiguousarray(bq[fsl].reshape(4, 128).T, np.float32),
            "lmask": lm,
            "gmask": gm_ones if hg == 0 else gm_band,
        })
    return in_maps


def kernel(x, wq, bq, wk, bk, wv, bv, wo, bo):
    from concourse.bass_utils import run_bass_kernel_spmd

    x, wq, bq, wk, bk, wv, bv, wo, bo = (
        np.asarray(a, np.float32) for a in (x, wq, bq, wk, bk, wv, bv, wo, bo))

    if "nc" not in _cache:
        _cache["nc"] = _build()
    nc = _cache["nc"]

    in_maps = _host_inputs(x, wq, bq, wk, bk, wv, bv, wo, bo)
    res = run_bass_kernel_spmd(nc, in_maps, core_ids=list(range(NCORES)))
    _cache["last_results"] = res

    const = (bo + bv @ wo.T).astype(np.float32)        # [1024]
    out = np.empty((B, N, E), np.float32)
    for b in range(B):
        out[b] = (np.asarray(res.results[2 * b]["out"], np.float32)
                  + np.asarray(res.results[2 * b + 1]["out"], np.float32)
                  + const)
    return out
